# revision 32
# baseline (speedup 1.0000x reference)
import numpy as np

B, S, H, NH, HD = 2, 4096, 512, 8, 64
N_CORES = 8

_CACHE = {}


def _emit(nc, tc, ctx, aps, S_, dbg=None):
    import concourse.mybir as mybir

    f32 = mybir.dt.float32
    bf16 = mybir.dt.bfloat16
    Exp = mybir.ActivationFunctionType.Exp
    mult = mybir.AluOpType.mult

    NQ = S_ // 512
    NK = S_ // 128
    NS = S_ // 128

    P = ctx.enter_context(tc.tile_pool(name="persist", bufs=1))

    wq_sb = P.tile([128, 512], bf16, tag="wq")
    nc.sync.dma_start(wq_sb[:], aps["wq"])
    wk_sb = P.tile([128, 512], bf16, tag="wk")
    nc.sync.dma_start(wk_sb[:], aps["wk"])
    wv_sb = P.tile([128, 512], bf16, tag="wv")
    nc.sync.dma_start(wv_sb[:], aps["wv"])
    wo_sb = P.tile([128, 512], bf16, tag="wo")
    nc.sync.dma_start(wo_sb[:], aps["wo"])
    bq_sb = P.tile([128, 1], f32, tag="bq")
    nc.sync.dma_start(bq_sb[:], aps["bq"])
    bk_sb = P.tile([128, 1], f32, tag="bk")
    nc.sync.dma_start(bk_sb[:], aps["bk"])

    qT = P.tile([128, S_], bf16, tag="qT")
    kT = P.tile([128, S_], bf16, tag="kT")
    Vt = P.tile([128, NK * 130], bf16, tag="V")
    ctxT = P.tile([128, S_], bf16, tag="ctxT")
    sumsT = [P.tile([128, NS], f32, tag=f"sumsT{h}", name=f"sumsT{h}")
             for h in (0, 1)]
    recT = [P.tile([128, NS], f32, tag=f"recT{h}", name=f"recT{h}")
            for h in (0, 1)]
    ones1 = P.tile([128, 1], f32, tag="ones1")
    nc.vector.memset(ones1[:], 1.0)
    vt_cols = Vt.rearrange("p (n c) -> p n c", c=65)
    nc.vector.tensor_copy(
        vt_cols[:, :, 64:65], ones1[:, 0:1].to_broadcast((128, 2 * NK, 1))
    )

    with tc.tile_pool(name="xt", bufs=1) as xtp, \
         tc.tile_pool(name="pj", bufs=2, space="PSUM") as pj:
        xts = []
        for j in range(4):
            t = xtp.tile([128, S_], bf16, tag=f"xt{j}")
            xts.append(t)
        for c in range(NQ):
            for j in range(4):
                nc.sync.dma_start(
                    xts[j][:, c * 512:(c + 1) * 512],
                    aps["xT"][j * 128:(j + 1) * 128, c * 512:(c + 1) * 512],
                )

        for wsb, bsb, dst in ((wk_sb, bk_sb, kT), (wq_sb, bq_sb, qT)):
            for c in range(NQ):
                ps = pj.tile([128, 512], f32, tag="pj")
                for j in range(4):
                    nc.tensor.matmul(
                        ps[:],
                        wsb[:, j * 128:(j + 1) * 128],
                        xts[j][:, c * 512:(c + 1) * 512],
                        start=(j == 0), stop=(j == 3),
                    )
                nc.vector.tensor_scalar_add(
                    dst[:, c * 512:(c + 1) * 512], ps[:], bsb[:, 0:1]
                )

        for st in range(NK):
            ps = pj.tile([128, 128], f32, tag="pj")
            for j in range(4):
                nc.tensor.matmul(
                    ps[:],
                    xts[j][:, st * 128:(st + 1) * 128],
                    wv_sb[:, j * 128:(j + 1) * 128],
                    start=(j == 0), stop=(j == 3),
                )
            nc.vector.tensor_copy(Vt[:, st * 130:st * 130 + 64], ps[:, 0:64])
            nc.vector.tensor_copy(Vt[:, st * 130 + 65:st * 130 + 129], ps[:, 64:128])

    items = [(h, t) for t in range(NK) for h in (0, 1)]
    groups = [items[i:i + 3] for i in range(0, len(items), 3)]

    with tc.tile_pool(name="sc", bufs=2, space="PSUM") as scp, \
         tc.tile_pool(name="cx", bufs=2, space="PSUM") as cxp, \
         tc.tile_pool(name="pb", bufs=3) as pbp, \
         tc.tile_pool(name="dr", bufs=4, space="DRAM") as drp, \
         tc.tile_pool(name="nm", bufs=4) as nmp:
        for c in range(NQ):
            cx = {h: cxp.tile([65, 512], f32, tag="cx", name=f"cx{h}")
                  for h in (0, 1)}
            for g in groups:
                n = len(g)
                sc = scp.tile([128, n * 512], f32, tag="sc")
                for i, (h, t) in enumerate(g):
                    nc.tensor.matmul(
                        sc[:, i * 512:(i + 1) * 512],
                        kT[h * 64:(h + 1) * 64, t * 128:(t + 1) * 128],
                        qT[h * 64:(h + 1) * 64, c * 512:(c + 1) * 512],
                        start=True, stop=True,
                    )
                pb = pbp.tile([128, n * 512], bf16, tag="pb")
                nc.scalar.activation(pb[:], sc[:], Exp, scale=1.0 / np.sqrt(HD))
                if dbg and c == 0 and g is groups[0]:
                    nc.sync.dma_start(dbg["pb0"], pb[:])
                for i, (h, t) in enumerate(g):
                    nc.tensor.matmul(
                        cx[h][:],
                        Vt[:, t * 130 + h * 65:t * 130 + (h + 1) * 65],
                        pb[:, i * 512:(i + 1) * 512],
                        start=(t == 0), stop=(t == NK - 1),
                    )
            for h in (0, 1):
                if dbg and c == 0:
                    cxd = nmp.tile([65, 512], f32, tag="cxd")
                    nc.vector.tensor_copy(cxd[:], cx[h][:])
                    nc.sync.dma_start(dbg[f"cx{h}"], cxd[:])
                sums = nmp.tile([1, 512], f32, tag="sums")
                nc.vector.tensor_copy(sums[0:1, :], cx[h][64:65, :])
                nc.vector.tensor_copy(
                    ctxT[h * 64:(h + 1) * 64, c * 512:(c + 1) * 512],
                    cx[h][0:64, :],
                )
                dsc = drp.tile([1, 512], f32, tag="dsums")
                nc.sync.dma_start(dsc[:], sums[0:1, :])
                nc.sync.dma_start(
                    sumsT[h][:, 4 * c:4 * c + 4],
                    dsc.rearrange("o (j p) -> (o p) j", p=128),
                )
                nc.vector.reciprocal_approx_fast(
                    recT[h][:, 4 * c:4 * c + 4], sumsT[h][:, 4 * c:4 * c + 4]
                )

    Ident = mybir.ActivationFunctionType.Identity
    with tc.tile_pool(name="op", bufs=4, space="PSUM") as opp, \
         tc.tile_pool(name="ob", bufs=3) as obp:
        for st in range(NS):
            ps0 = opp.tile([128, 512], f32, tag="op", name="ps0")
            nc.tensor.matmul(
                ps0[:],
                ctxT[0:64, st * 128:(st + 1) * 128],
                wo_sb[0:64, :],
                start=True, stop=True,
            )
            ps1 = opp.tile([128, 512], f32, tag="op", name="ps1")
            nc.tensor.matmul(
                ps1[:],
                ctxT[64:128, st * 128:(st + 1) * 128],
                wo_sb[64:128, :],
                start=True, stop=True,
            )
            tmp = obp.tile([128, 512], f32, tag="tmp")
            nc.vector.tensor_scalar_mul(tmp[:], ps0[:], recT[0][:, st:st + 1])
            ob = obp.tile([128, 512], f32, tag="ob")
            nc.vector.scalar_tensor_tensor(
                ob[:], ps1[:], recT[1][:, st:st + 1], tmp[:],
                op0=mult, op1=mybir.AluOpType.add,
            )
            nc.sync.dma_start(aps["outp"][st * 128:(st + 1) * 128, :], ob[:])

    if dbg:
        nc.sync.dma_start(dbg["qT"], qT[:])
        nc.sync.dma_start(dbg["kT"], kT[:])
        nc.sync.dma_start(dbg["Vt"], Vt[:])
        nc.sync.dma_start(dbg["ctxT0"], ctxT[0:64, :])
        nc.sync.dma_start(dbg["ctxT1"], ctxT[64:128, :])


def _build(S_=S, debug_dump=False):
    from contextlib import ExitStack

    import concourse.mybir as mybir
    import concourse.tile as tile
    from concourse import bacc

    f32 = mybir.dt.float32
    bf16 = mybir.dt.bfloat16
    nc = bacc.Bacc("TRN2", target_bir_lowering=False, debug=False,
                   num_devices=N_CORES)
    aps = {
        "xT": nc.dram_tensor("xT", [H, S_], bf16, kind="ExternalInput").ap(),
        "wq": nc.dram_tensor("wq", [128, H], bf16, kind="ExternalInput").ap(),
        "wk": nc.dram_tensor("wk", [128, H], bf16, kind="ExternalInput").ap(),
        "wv": nc.dram_tensor("wv", [128, H], bf16, kind="ExternalInput").ap(),
        "wo": nc.dram_tensor("wo", [128, H], bf16, kind="ExternalInput").ap(),
        "bq": nc.dram_tensor("bq", [128, 1], f32, kind="ExternalInput").ap(),
        "bk": nc.dram_tensor("bk", [128, 1], f32, kind="ExternalInput").ap(),
        "outp": nc.dram_tensor("outp", [S_, H], f32, kind="ExternalOutput").ap(),
    }
    dbg = None
    if debug_dump:
        NK = S_ // 128
        dbg = {
            "qT": nc.dram_tensor("d_qT", [128, S_], bf16,
                                 kind="ExternalOutput").ap(),
            "kT": nc.dram_tensor("d_kT", [128, S_], bf16,
                                 kind="ExternalOutput").ap(),
            "Vt": nc.dram_tensor("d_Vt", [128, NK * 130], bf16,
                                 kind="ExternalOutput").ap(),
            "ctxT0": nc.dram_tensor("d_ctxT0", [64, S_], bf16,
                                    kind="ExternalOutput").ap(),
            "ctxT1": nc.dram_tensor("d_ctxT1", [64, S_], bf16,
                                    kind="ExternalOutput").ap(),
            "pb0": nc.dram_tensor("d_pb0", [128, 3 * 512], bf16,
                                  kind="ExternalOutput").ap(),
            "cx0": nc.dram_tensor("d_cx0", [65, 512], f32,
                                  kind="ExternalOutput").ap(),
            "cx1": nc.dram_tensor("d_cx1", [65, 512], f32,
                                  kind="ExternalOutput").ap(),
        }
    with tile.TileContext(nc) as tc:
        with ExitStack() as stack:
            _emit(nc, tc, stack, aps, S_, dbg=dbg)
    nc.compile()
    return nc


def _rearrange_w(wT_slice):
    import ml_dtypes

    return np.ascontiguousarray(
        wT_slice.reshape(4, 128, 128).transpose(1, 0, 2).reshape(128, 512)
    ).astype(ml_dtypes.bfloat16)


def _host_prep(hidden_states, Wq, bq, Wk, bk, Wv, bv, Wo, bo, S_=S):
    import ml_dtypes

    bf = ml_dtypes.bfloat16
    xT = [np.ascontiguousarray(hidden_states[b].T).astype(bf)
          for b in range(B)]
    in_maps = []
    for c in range(N_CORES):
        b, p = c // 4, c % 4
        sl = slice(p * 128, (p + 1) * 128)
        in_maps.append({
            "xT": xT[b],
            "wq": _rearrange_w(Wq.T[:, sl].astype(np.float32)),
            "wk": _rearrange_w(Wk.T[:, sl].astype(np.float32)),
            "wv": _rearrange_w(Wv.T[:, sl].astype(np.float32)),
            "wo": np.ascontiguousarray(
                Wo.T[p * 128:(p + 1) * 128, :]).astype(bf),
            "bq": np.ascontiguousarray(
                bq[sl].astype(np.float32).reshape(128, 1)),
            "bk": np.ascontiguousarray(
                bk[sl].astype(np.float32).reshape(128, 1)),
        })
    return in_maps


def kernel(hidden_states, Wq, bq, Wk, bk, Wv, bv, Wo, bo):
    from concourse.bass_utils import run_bass_kernel_spmd

    hidden_states = np.asarray(hidden_states)
    Wq, bq = np.asarray(Wq), np.asarray(bq)
    Wk, bk = np.asarray(Wk), np.asarray(bk)
    Wv, bv = np.asarray(Wv), np.asarray(bv)
    Wo, bo = np.asarray(Wo), np.asarray(bo)

    if "nc" not in _CACHE:
        _CACHE["nc"] = _build(S)
    nc = _CACHE["nc"]

    in_maps = _host_prep(hidden_states, Wq, bq, Wk, bk, Wv, bv, Wo, bo)
    res = run_bass_kernel_spmd(nc, in_maps, core_ids=list(range(N_CORES)))

    bo_eff = (bo.astype(np.float64) +
              bv.astype(np.float64) @ Wo.T.astype(np.float64)).astype(np.float32)
    out = np.empty((B, S, H), dtype=np.float32)
    for b in range(B):
        acc = np.zeros((S, H), dtype=np.float32)
        for p in range(4):
            acc += res.results[b * 4 + p]["outp"]
        out[b] = acc + bo_eff
    return out


# revision 34
# speedup vs baseline: 1.0771x; 1.0771x over previous
import numpy as np

B, S, H, NH, HD = 2, 4096, 512, 8, 64
N_CORES = 8

_CACHE = {}


def _emit(nc, tc, ctx, aps, S_, dbg=None):
    import concourse.mybir as mybir

    f32 = mybir.dt.float32
    bf16 = mybir.dt.bfloat16
    Exp = mybir.ActivationFunctionType.Exp
    mult = mybir.AluOpType.mult
    add = mybir.AluOpType.add

    NQ = S_ // 512
    NK = S_ // 128
    NS = S_ // 128

    P = ctx.enter_context(tc.tile_pool(name="persist", bufs=1))

    wq_sb = P.tile([128, 512], bf16, tag="wq")
    nc.sync.dma_start(wq_sb[:], aps["wq"])
    wk_sb = P.tile([128, 512], bf16, tag="wk")
    nc.sync.dma_start(wk_sb[:], aps["wk"])
    wv_sb = P.tile([128, 512], bf16, tag="wv")
    nc.sync.dma_start(wv_sb[:], aps["wv"])
    wo_sb = P.tile([128, 512], bf16, tag="wo")
    nc.sync.dma_start(wo_sb[:], aps["wo"])
    bq_sb = P.tile([128, 1], f32, tag="bq")
    nc.sync.dma_start(bq_sb[:], aps["bq"])
    bk_sb = P.tile([128, 1], f32, tag="bk")
    nc.sync.dma_start(bk_sb[:], aps["bk"])

    qT = P.tile([128, S_], bf16, tag="qT")
    kT = P.tile([128, S_], bf16, tag="kT")
    Vt = P.tile([128, NK * 130], bf16, tag="V")
    ctxT = P.tile([128, S_], bf16, tag="ctxT")
    sumsT = [P.tile([128, NS], f32, tag=f"sumsT{h}", name=f"sumsT{h}")
             for h in (0, 1)]
    recT = [P.tile([128, NS], f32, tag=f"recT{h}", name=f"recT{h}")
            for h in (0, 1)]
    ones1 = P.tile([128, 1], f32, tag="ones1")
    nc.vector.memset(ones1[:], 1.0)
    vt_cols = Vt.rearrange("p (n c) -> p n c", c=65)
    nc.vector.tensor_copy(
        vt_cols[:, :, 64:65], ones1[:, 0:1].to_broadcast((128, 2 * NK, 1))
    )

    nmp = ctx.enter_context(tc.tile_pool(name="nm", bufs=4))
    drp = ctx.enter_context(tc.tile_pool(name="dr", bufs=4, space="DRAM"))
    pbp = ctx.enter_context(tc.tile_pool(name="pb", bufs=3))
    cxp = ctx.enter_context(tc.tile_pool(name="cx", bufs=2, space="PSUM"))

    def scores_mm(sc_slice, h, t, c):
        nc.tensor.matmul(
            sc_slice,
            kT[h * 64:(h + 1) * 64, t * 128:(t + 1) * 128],
            qT[h * 64:(h + 1) * 64, c * 512:(c + 1) * 512],
            start=True, stop=True,
        )

    def pv_mm(cxt, h, t, pb_slice):
        nc.tensor.matmul(
            cxt,
            Vt[:, t * 130 + h * 65:t * 130 + (h + 1) * 65],
            pb_slice,
            start=(t == 0), stop=(t == NK - 1),
        )

    def evac(cxm, h, c):
        sums = nmp.tile([1, 512], f32, tag="sums", name="sums")
        nc.vector.tensor_copy(sums[0:1, :], cxm[64:65, :])
        nc.vector.tensor_copy(
            ctxT[h * 64:(h + 1) * 64, c * 512:(c + 1) * 512], cxm[0:64, :]
        )
        dsc = drp.tile([1, 512], f32, tag="dsums", name="dsc")
        nc.sync.dma_start(dsc[:], sums[0:1, :])
        nc.sync.dma_start(
            sumsT[h][:, 4 * c:4 * c + 4],
            dsc.rearrange("o (j p) -> (o p) j", p=128),
        )
        nc.vector.reciprocal_approx_fast(
            recT[h][:, 4 * c:4 * c + 4], sumsT[h][:, 4 * c:4 * c + 4]
        )

    with tc.tile_pool(name="xt", bufs=1) as xtp, \
         tc.tile_pool(name="pj", bufs=2, space="PSUM") as pj, \
         tc.tile_pool(name="s2", bufs=2, space="PSUM") as s2p:
        xts = []
        for j in range(4):
            t = xtp.tile([128, S_], bf16, tag=f"xt{j}")
            xts.append(t)
        for c in range(NQ):
            for j in range(4):
                nc.sync.dma_start(
                    xts[j][:, c * 512:(c + 1) * 512],
                    aps["xT"][j * 128:(j + 1) * 128, c * 512:(c + 1) * 512],
                )

        def kq_proj_chunk(wsb, bsb, dst, c):
            ps = pj.tile([128, 512], f32, tag="pj", name="pjt")
            for j in range(4):
                nc.tensor.matmul(
                    ps[:],
                    wsb[:, j * 128:(j + 1) * 128],
                    xts[j][:, c * 512:(c + 1) * 512],
                    start=(j == 0), stop=(j == 3),
                )
            nc.vector.tensor_scalar_add(
                dst[:, c * 512:(c + 1) * 512], ps[:], bsb[:, 0:1]
            )

        def v_proj_tile(t):
            ps = pj.tile([128, 128], f32, tag="pj", name="pjv")
            for j in range(4):
                nc.tensor.matmul(
                    ps[:],
                    xts[j][:, t * 128:(t + 1) * 128],
                    wv_sb[:, j * 128:(j + 1) * 128],
                    start=(j == 0), stop=(j == 3),
                )
            nc.vector.tensor_copy(Vt[:, t * 130:t * 130 + 64], ps[:, 0:64])
            nc.vector.tensor_copy(
                Vt[:, t * 130 + 65:t * 130 + 129], ps[:, 64:128]
            )

        kq_proj_chunk(wk_sb, bk_sb, kT, 0)
        kq_proj_chunk(wq_sb, bq_sb, qT, 0)
        NVUP = min(6, NK)
        for t in range(NVUP):
            v_proj_tile(t)

        tasks = ([("k", c) for c in range(1, NQ)]
                 + [("v", t) for t in range(NVUP, NK)]
                 + [("q", c) for c in range(1, NQ)])
        ti = 0

        cx0 = {h: cxp.tile([65, 512], f32, tag="cx", name=f"cx{h}")
               for h in (0, 1)}
        for t in range(NK):
            sc = s2p.tile([128, 1024], f32, tag="s2", name="s2")
            scores_mm(sc[:, 0:512], 0, t, 0)
            scores_mm(sc[:, 512:1024], 1, t, 0)
            pb = pbp.tile([128, 1024], bf16, tag="pb", name="pb")
            nc.scalar.activation(pb[:], sc[:], Exp, scale=1.0 / np.sqrt(HD))
            if dbg and t == 0:
                nc.sync.dma_start(dbg["pb0"], pb[:])
            while ti * NK < (t + 1) * len(tasks):
                kind, v = tasks[ti]
                ti += 1
                if kind == "k":
                    kq_proj_chunk(wk_sb, bk_sb, kT, v)
                elif kind == "q":
                    kq_proj_chunk(wq_sb, bq_sb, qT, v)
                else:
                    v_proj_tile(v)
            pv_mm(cx0[0][:], 0, t, pb[:, 0:512])
            pv_mm(cx0[1][:], 1, t, pb[:, 512:1024])
        for h in (0, 1):
            if dbg:
                cxd = nmp.tile([65, 512], f32, tag="cxd")
                nc.vector.tensor_copy(cxd[:], cx0[h][:])
                nc.sync.dma_start(dbg[f"cx{h}"], cxd[:])
            evac(cx0[h], h, 0)

    items = [(h, t) for t in range(NK) for h in (0, 1)]
    groups = [items[i:i + 3] for i in range(0, len(items), 3)]
    with tc.tile_pool(name="sc", bufs=2, space="PSUM") as scp:
        for c in range(1, NQ):
            cx = {h: cxp.tile([65, 512], f32, tag="cx", name=f"cx{h}")
                  for h in (0, 1)}
            for g in groups:
                n = len(g)
                sc = scp.tile([128, n * 512], f32, tag="sc")
                for i, (h, t) in enumerate(g):
                    scores_mm(sc[:, i * 512:(i + 1) * 512], h, t, c)
                pb = pbp.tile([128, n * 512], bf16, tag="pb", name="pb")
                nc.scalar.activation(pb[:], sc[:], Exp, scale=1.0 / np.sqrt(HD))
                for i, (h, t) in enumerate(g):
                    pv_mm(cx[h][:], h, t, pb[:, i * 512:(i + 1) * 512])
            for h in (0, 1):
                evac(cx[h], h, c)

    with tc.tile_pool(name="op", bufs=4, space="PSUM") as opp, \
         tc.tile_pool(name="ob", bufs=3) as obp:
        for st in range(NS):
            ps0 = opp.tile([128, 512], f32, tag="op", name="ps0")
            nc.tensor.matmul(
                ps0[:],
                ctxT[0:64, st * 128:(st + 1) * 128],
                wo_sb[0:64, :],
                start=True, stop=True,
            )
            ps1 = opp.tile([128, 512], f32, tag="op", name="ps1")
            nc.tensor.matmul(
                ps1[:],
                ctxT[64:128, st * 128:(st + 1) * 128],
                wo_sb[64:128, :],
                start=True, stop=True,
            )
            tmp = obp.tile([128, 512], f32, tag="tmp")
            nc.vector.tensor_scalar_mul(tmp[:], ps0[:], recT[0][:, st:st + 1])
            ob = obp.tile([128, 512], f32, tag="ob")
            nc.vector.scalar_tensor_tensor(
                ob[:], ps1[:], recT[1][:, st:st + 1], tmp[:],
                op0=mult, op1=add,
            )
            nc.sync.dma_start(aps["outp"][st * 128:(st + 1) * 128, :], ob[:])

    if dbg:
        nc.sync.dma_start(dbg["qT"], qT[:])
        nc.sync.dma_start(dbg["kT"], kT[:])
        nc.sync.dma_start(dbg["Vt"], Vt[:])
        nc.sync.dma_start(dbg["ctxT0"], ctxT[0:64, :])
        nc.sync.dma_start(dbg["ctxT1"], ctxT[64:128, :])


def _build(S_=S, debug_dump=False):
    from contextlib import ExitStack

    import concourse.mybir as mybir
    import concourse.tile as tile
    from concourse import bacc

    f32 = mybir.dt.float32
    bf16 = mybir.dt.bfloat16
    nc = bacc.Bacc("TRN2", target_bir_lowering=False, debug=False,
                   num_devices=N_CORES)
    aps = {
        "xT": nc.dram_tensor("xT", [H, S_], bf16, kind="ExternalInput").ap(),
        "wq": nc.dram_tensor("wq", [128, H], bf16, kind="ExternalInput").ap(),
        "wk": nc.dram_tensor("wk", [128, H], bf16, kind="ExternalInput").ap(),
        "wv": nc.dram_tensor("wv", [128, H], bf16, kind="ExternalInput").ap(),
        "wo": nc.dram_tensor("wo", [128, H], bf16, kind="ExternalInput").ap(),
        "bq": nc.dram_tensor("bq", [128, 1], f32, kind="ExternalInput").ap(),
        "bk": nc.dram_tensor("bk", [128, 1], f32, kind="ExternalInput").ap(),
        "outp": nc.dram_tensor("outp", [S_, H], f32, kind="ExternalOutput").ap(),
    }
    dbg = None
    if debug_dump:
        NK = S_ // 128
        dbg = {
            "qT": nc.dram_tensor("d_qT", [128, S_], bf16,
                                 kind="ExternalOutput").ap(),
            "kT": nc.dram_tensor("d_kT", [128, S_], bf16,
                                 kind="ExternalOutput").ap(),
            "Vt": nc.dram_tensor("d_Vt", [128, NK * 130], bf16,
                                 kind="ExternalOutput").ap(),
            "ctxT0": nc.dram_tensor("d_ctxT0", [64, S_], bf16,
                                    kind="ExternalOutput").ap(),
            "ctxT1": nc.dram_tensor("d_ctxT1", [64, S_], bf16,
                                    kind="ExternalOutput").ap(),
            "pb0": nc.dram_tensor("d_pb0", [128, 2 * 512], bf16,
                                  kind="ExternalOutput").ap(),
            "cx0": nc.dram_tensor("d_cx0", [65, 512], f32,
                                  kind="ExternalOutput").ap(),
            "cx1": nc.dram_tensor("d_cx1", [65, 512], f32,
                                  kind="ExternalOutput").ap(),
        }
    with tile.TileContext(nc) as tc:
        with ExitStack() as stack:
            _emit(nc, tc, stack, aps, S_, dbg=dbg)
    nc.compile()
    return nc


def _rearrange_w(wT_slice):
    import ml_dtypes

    return np.ascontiguousarray(
        wT_slice.reshape(4, 128, 128).transpose(1, 0, 2).reshape(128, 512)
    ).astype(ml_dtypes.bfloat16)


def _host_prep(hidden_states, Wq, bq, Wk, bk, Wv, bv, Wo, bo, S_=S):
    import ml_dtypes

    bf = ml_dtypes.bfloat16
    xT = [np.ascontiguousarray(hidden_states[b].T).astype(bf)
          for b in range(B)]
    in_maps = []
    for c in range(N_CORES):
        b, p = c // 4, c % 4
        sl = slice(p * 128, (p + 1) * 128)
        in_maps.append({
            "xT": xT[b],
            "wq": _rearrange_w(Wq.T[:, sl].astype(np.float32)),
            "wk": _rearrange_w(Wk.T[:, sl].astype(np.float32)),
            "wv": _rearrange_w(Wv.T[:, sl].astype(np.float32)),
            "wo": np.ascontiguousarray(
                Wo.T[p * 128:(p + 1) * 128, :]).astype(bf),
            "bq": np.ascontiguousarray(
                bq[sl].astype(np.float32).reshape(128, 1)),
            "bk": np.ascontiguousarray(
                bk[sl].astype(np.float32).reshape(128, 1)),
        })
    return in_maps


def kernel(hidden_states, Wq, bq, Wk, bk, Wv, bv, Wo, bo):
    from concourse.bass_utils import run_bass_kernel_spmd

    hidden_states = np.asarray(hidden_states)
    Wq, bq = np.asarray(Wq), np.asarray(bq)
    Wk, bk = np.asarray(Wk), np.asarray(bk)
    Wv, bv = np.asarray(Wv), np.asarray(bv)
    Wo, bo = np.asarray(Wo), np.asarray(bo)

    if "nc" not in _CACHE:
        _CACHE["nc"] = _build(S)
    nc = _CACHE["nc"]

    in_maps = _host_prep(hidden_states, Wq, bq, Wk, bk, Wv, bv, Wo, bo)
    res = run_bass_kernel_spmd(nc, in_maps, core_ids=list(range(N_CORES)))

    bo_eff = (bo.astype(np.float64) +
              bv.astype(np.float64) @ Wo.T.astype(np.float64)).astype(np.float32)
    out = np.empty((B, S, H), dtype=np.float32)
    for b in range(B):
        acc = np.zeros((S, H), dtype=np.float32)
        for p in range(4):
            acc += res.results[b * 4 + p]["outp"]
        out[b] = acc + bo_eff
    return out


# revision 35
# speedup vs baseline: 1.0837x; 1.0061x over previous
import numpy as np

B, S, H, NH, HD = 2, 4096, 512, 8, 64
N_CORES = 8

_CACHE = {}


def _emit(nc, tc, ctx, aps, S_, dbg=None):
    import concourse.mybir as mybir

    f32 = mybir.dt.float32
    bf16 = mybir.dt.bfloat16
    Exp = mybir.ActivationFunctionType.Exp
    mult = mybir.AluOpType.mult
    add = mybir.AluOpType.add

    NQ = S_ // 512
    NK = S_ // 128
    NS = S_ // 128

    P = ctx.enter_context(tc.tile_pool(name="persist", bufs=1))

    wq_sb = P.tile([128, 512], bf16, tag="wq")
    nc.sync.dma_start(wq_sb[:], aps["wq"])
    wk_sb = P.tile([128, 512], bf16, tag="wk")
    nc.sync.dma_start(wk_sb[:], aps["wk"])
    wv_sb = P.tile([128, 512], bf16, tag="wv")
    nc.sync.dma_start(wv_sb[:], aps["wv"])
    wo_sb = P.tile([128, 512], bf16, tag="wo")
    nc.sync.dma_start(wo_sb[:], aps["wo"])
    bq_sb = P.tile([128, 1], f32, tag="bq")
    nc.sync.dma_start(bq_sb[:], aps["bq"])
    bk_sb = P.tile([128, 1], f32, tag="bk")
    nc.sync.dma_start(bk_sb[:], aps["bk"])

    qT = P.tile([128, S_], bf16, tag="qT")
    kT = P.tile([128, S_], bf16, tag="kT")
    Vt = P.tile([128, NK * 130], bf16, tag="V")
    ctxT = P.tile([128, S_], bf16, tag="ctxT")
    sumsT = [P.tile([128, NS], f32, tag=f"sumsT{h}", name=f"sumsT{h}")
             for h in (0, 1)]
    recT = [P.tile([128, NS], f32, tag=f"recT{h}", name=f"recT{h}")
            for h in (0, 1)]
    ones1 = P.tile([128, 1], f32, tag="ones1")
    nc.vector.memset(ones1[:], 1.0)
    vt_cols = Vt.rearrange("p (n c) -> p n c", c=65)
    nc.vector.tensor_copy(
        vt_cols[:, :, 64:65], ones1[:, 0:1].to_broadcast((128, 2 * NK, 1))
    )

    nmp = ctx.enter_context(tc.tile_pool(name="nm", bufs=4))
    drp = ctx.enter_context(tc.tile_pool(name="dr", bufs=4, space="DRAM"))
    pbp = ctx.enter_context(tc.tile_pool(name="pb", bufs=3))
    cxp = ctx.enter_context(tc.tile_pool(name="cx", bufs=2, space="PSUM"))

    def scores_mm(sc_slice, h, t, c):
        nc.tensor.matmul(
            sc_slice,
            kT[h * 64:(h + 1) * 64, t * 128:(t + 1) * 128],
            qT[h * 64:(h + 1) * 64, c * 512:(c + 1) * 512],
            start=True, stop=True,
        )

    def pv_mm(cxt, h, t, pb_slice):
        nc.tensor.matmul(
            cxt,
            Vt[:, t * 130 + h * 65:t * 130 + (h + 1) * 65],
            pb_slice,
            start=(t == 0), stop=(t == NK - 1),
        )

    def evac(cxm, h, c):
        sums = nmp.tile([1, 512], f32, tag="sums", name="sums")
        nc.vector.tensor_copy(sums[0:1, :], cxm[64:65, :])
        nc.vector.tensor_copy(
            ctxT[h * 64:(h + 1) * 64, c * 512:(c + 1) * 512], cxm[0:64, :]
        )
        dsc = drp.tile([1, 512], f32, tag="dsums", name="dsc")
        nc.sync.dma_start(dsc[:], sums[0:1, :])
        nc.sync.dma_start(
            sumsT[h][:, 4 * c:4 * c + 4],
            dsc.rearrange("o (j p) -> (o p) j", p=128),
        )
        nc.vector.reciprocal_approx_fast(
            recT[h][:, 4 * c:4 * c + 4], sumsT[h][:, 4 * c:4 * c + 4]
        )

    with tc.tile_pool(name="xt", bufs=1) as xtp, \
         tc.tile_pool(name="pj", bufs=2, space="PSUM") as pj, \
         tc.tile_pool(name="s2", bufs=2, space="PSUM") as s2p:
        xts = []
        for j in range(4):
            t = xtp.tile([128, S_], bf16, tag=f"xt{j}")
            xts.append(t)
        for c in range(NQ):
            for j in range(4):
                nc.sync.dma_start(
                    xts[j][:, c * 512:(c + 1) * 512],
                    aps["xT"][j * 128:(j + 1) * 128, c * 512:(c + 1) * 512],
                )

        def kq_proj_chunk(wsb, bsb, dst, c):
            ps = pj.tile([128, 512], f32, tag="pj", name="pjt")
            for j in range(4):
                nc.tensor.matmul(
                    ps[:],
                    wsb[:, j * 128:(j + 1) * 128],
                    xts[j][:, c * 512:(c + 1) * 512],
                    start=(j == 0), stop=(j == 3),
                )
            nc.vector.tensor_scalar_add(
                dst[:, c * 512:(c + 1) * 512], ps[:], bsb[:, 0:1]
            )

        def v_proj_tile(t):
            ps = pj.tile([128, 128], f32, tag="pj", name="pjv")
            for j in range(4):
                nc.tensor.matmul(
                    ps[:],
                    xts[j][:, t * 128:(t + 1) * 128],
                    wv_sb[:, j * 128:(j + 1) * 128],
                    start=(j == 0), stop=(j == 3),
                )
            nc.vector.tensor_copy(Vt[:, t * 130:t * 130 + 64], ps[:, 0:64])
            nc.vector.tensor_copy(
                Vt[:, t * 130 + 65:t * 130 + 129], ps[:, 64:128]
            )

        kq_proj_chunk(wk_sb, bk_sb, kT, 0)
        kq_proj_chunk(wq_sb, bq_sb, qT, 0)
        NVUP = min(6, NK)
        for t in range(NVUP):
            v_proj_tile(t)

        tasks = ([("k", c) for c in range(1, NQ)]
                 + [("v", t) for t in range(NVUP, NK)]
                 + [("q", c) for c in range(1, NQ)])
        ti = 0

        cx0 = {h: cxp.tile([65, 512], f32, tag="cx", name=f"cx{h}")
               for h in (0, 1)}
        for t in range(NK):
            sc = s2p.tile([128, 1024], f32, tag="s2", name="s2")
            scores_mm(sc[:, 0:512], 0, t, 0)
            scores_mm(sc[:, 512:1024], 1, t, 0)
            pb = pbp.tile([128, 1024], bf16, tag="pb", name="pb")
            nc.scalar.activation(pb[:], sc[:], Exp, scale=1.0 / np.sqrt(HD))
            if dbg and t == 0:
                nc.sync.dma_start(dbg["pb0"], pb[:])
            while ti * NK < (t + 1) * len(tasks):
                kind, v = tasks[ti]
                ti += 1
                if kind == "k":
                    kq_proj_chunk(wk_sb, bk_sb, kT, v)
                elif kind == "q":
                    kq_proj_chunk(wq_sb, bq_sb, qT, v)
                else:
                    v_proj_tile(v)
            pv_mm(cx0[0][:], 0, t, pb[:, 0:512])
            pv_mm(cx0[1][:], 1, t, pb[:, 512:1024])
        for h in (0, 1):
            if dbg:
                cxd = nmp.tile([65, 512], f32, tag="cxd")
                nc.vector.tensor_copy(cxd[:], cx0[h][:])
                nc.sync.dma_start(dbg[f"cx{h}"], cxd[:])
            evac(cx0[h], h, 0)

    items = [(h, t) for t in range(NK) for h in (0, 1)]
    groups = [items[i:i + 3] for i in range(0, len(items), 3)]
    with tc.tile_pool(name="sc", bufs=2, space="PSUM") as scp:
        for c in range(1, NQ):
            cx = {h: cxp.tile([65, 512], f32, tag="cx", name=f"cx{h}")
                  for h in (0, 1)}
            for g in groups:
                n = len(g)
                sc = scp.tile([128, n * 512], f32, tag="sc")
                for i, (h, t) in enumerate(g):
                    scores_mm(sc[:, i * 512:(i + 1) * 512], h, t, c)
                pb = pbp.tile([128, n * 512], bf16, tag="pb", name="pb")
                nc.scalar.activation(pb[:], sc[:], Exp, scale=1.0 / np.sqrt(HD))
                for i, (h, t) in enumerate(g):
                    pv_mm(cx[h][:], h, t, pb[:, i * 512:(i + 1) * 512])
            for h in (0, 1):
                evac(cx[h], h, c)

    with tc.tile_pool(name="op", bufs=4, space="PSUM") as opp, \
         tc.tile_pool(name="ob", bufs=3) as obp:
        for st in range(NS):
            ps0 = opp.tile([128, 512], f32, tag="op", name="ps0")
            nc.tensor.matmul(
                ps0[:],
                ctxT[0:64, st * 128:(st + 1) * 128],
                wo_sb[0:64, :],
                start=True, stop=True,
            )
            ps1 = opp.tile([128, 512], f32, tag="op", name="ps1")
            nc.tensor.matmul(
                ps1[:],
                ctxT[64:128, st * 128:(st + 1) * 128],
                wo_sb[64:128, :],
                start=True, stop=True,
            )
            tmp = obp.tile([128, 512], f32, tag="tmp")
            nc.scalar.activation(tmp[:], ps0[:],
                                 mybir.ActivationFunctionType.Identity,
                                 scale=recT[0][:, st:st + 1])
            ob = obp.tile([128, 512], f32, tag="ob")
            nc.vector.scalar_tensor_tensor(
                ob[:], ps1[:], recT[1][:, st:st + 1], tmp[:],
                op0=mult, op1=add,
            )
            nc.sync.dma_start(aps["outp"][st * 128:(st + 1) * 128, :], ob[:])

    if dbg:
        nc.sync.dma_start(dbg["qT"], qT[:])
        nc.sync.dma_start(dbg["kT"], kT[:])
        nc.sync.dma_start(dbg["Vt"], Vt[:])
        nc.sync.dma_start(dbg["ctxT0"], ctxT[0:64, :])
        nc.sync.dma_start(dbg["ctxT1"], ctxT[64:128, :])


def _build(S_=S, debug_dump=False):
    from contextlib import ExitStack

    import concourse.mybir as mybir
    import concourse.tile as tile
    from concourse import bacc

    f32 = mybir.dt.float32
    bf16 = mybir.dt.bfloat16
    nc = bacc.Bacc("TRN2", target_bir_lowering=False, debug=False,
                   num_devices=N_CORES)
    aps = {
        "xT": nc.dram_tensor("xT", [H, S_], bf16, kind="ExternalInput").ap(),
        "wq": nc.dram_tensor("wq", [128, H], bf16, kind="ExternalInput").ap(),
        "wk": nc.dram_tensor("wk", [128, H], bf16, kind="ExternalInput").ap(),
        "wv": nc.dram_tensor("wv", [128, H], bf16, kind="ExternalInput").ap(),
        "wo": nc.dram_tensor("wo", [128, H], bf16, kind="ExternalInput").ap(),
        "bq": nc.dram_tensor("bq", [128, 1], f32, kind="ExternalInput").ap(),
        "bk": nc.dram_tensor("bk", [128, 1], f32, kind="ExternalInput").ap(),
        "outp": nc.dram_tensor("outp", [S_, H], f32, kind="ExternalOutput").ap(),
    }
    dbg = None
    if debug_dump:
        NK = S_ // 128
        dbg = {
            "qT": nc.dram_tensor("d_qT", [128, S_], bf16,
                                 kind="ExternalOutput").ap(),
            "kT": nc.dram_tensor("d_kT", [128, S_], bf16,
                                 kind="ExternalOutput").ap(),
            "Vt": nc.dram_tensor("d_Vt", [128, NK * 130], bf16,
                                 kind="ExternalOutput").ap(),
            "ctxT0": nc.dram_tensor("d_ctxT0", [64, S_], bf16,
                                    kind="ExternalOutput").ap(),
            "ctxT1": nc.dram_tensor("d_ctxT1", [64, S_], bf16,
                                    kind="ExternalOutput").ap(),
            "pb0": nc.dram_tensor("d_pb0", [128, 2 * 512], bf16,
                                  kind="ExternalOutput").ap(),
            "cx0": nc.dram_tensor("d_cx0", [65, 512], f32,
                                  kind="ExternalOutput").ap(),
            "cx1": nc.dram_tensor("d_cx1", [65, 512], f32,
                                  kind="ExternalOutput").ap(),
        }
    with tile.TileContext(nc) as tc:
        with ExitStack() as stack:
            _emit(nc, tc, stack, aps, S_, dbg=dbg)
    nc.compile()
    return nc


def _rearrange_w(wT_slice):
    import ml_dtypes

    return np.ascontiguousarray(
        wT_slice.reshape(4, 128, 128).transpose(1, 0, 2).reshape(128, 512)
    ).astype(ml_dtypes.bfloat16)


def _host_prep(hidden_states, Wq, bq, Wk, bk, Wv, bv, Wo, bo, S_=S):
    import ml_dtypes

    bf = ml_dtypes.bfloat16
    xT = [np.ascontiguousarray(hidden_states[b].T).astype(bf)
          for b in range(B)]
    in_maps = []
    for c in range(N_CORES):
        b, p = c // 4, c % 4
        sl = slice(p * 128, (p + 1) * 128)
        in_maps.append({
            "xT": xT[b],
            "wq": _rearrange_w(Wq.T[:, sl].astype(np.float32)),
            "wk": _rearrange_w(Wk.T[:, sl].astype(np.float32)),
            "wv": _rearrange_w(Wv.T[:, sl].astype(np.float32)),
            "wo": np.ascontiguousarray(
                Wo.T[p * 128:(p + 1) * 128, :]).astype(bf),
            "bq": np.ascontiguousarray(
                bq[sl].astype(np.float32).reshape(128, 1)),
            "bk": np.ascontiguousarray(
                bk[sl].astype(np.float32).reshape(128, 1)),
        })
    return in_maps


def kernel(hidden_states, Wq, bq, Wk, bk, Wv, bv, Wo, bo):
    from concourse.bass_utils import run_bass_kernel_spmd

    hidden_states = np.asarray(hidden_states)
    Wq, bq = np.asarray(Wq), np.asarray(bq)
    Wk, bk = np.asarray(Wk), np.asarray(bk)
    Wv, bv = np.asarray(Wv), np.asarray(bv)
    Wo, bo = np.asarray(Wo), np.asarray(bo)

    if "nc" not in _CACHE:
        _CACHE["nc"] = _build(S)
    nc = _CACHE["nc"]

    in_maps = _host_prep(hidden_states, Wq, bq, Wk, bk, Wv, bv, Wo, bo)
    res = run_bass_kernel_spmd(nc, in_maps, core_ids=list(range(N_CORES)))

    bo_eff = (bo.astype(np.float64) +
              bv.astype(np.float64) @ Wo.T.astype(np.float64)).astype(np.float32)
    out = np.empty((B, S, H), dtype=np.float32)
    for b in range(B):
        acc = np.zeros((S, H), dtype=np.float32)
        for p in range(4):
            acc += res.results[b * 4 + p]["outp"]
        out[b] = acc + bo_eff
    return out


# revision 41
# speedup vs baseline: 1.1359x; 1.0482x over previous
import numpy as np

B, S, H, NH, HD = 2, 4096, 512, 8, 64
N_CORES = 8

_CACHE = {}


def _emit(nc, tc, ctx, aps, S_, dbg=None):
    import concourse.mybir as mybir

    f32 = mybir.dt.float32
    bf16 = mybir.dt.bfloat16
    Exp = mybir.ActivationFunctionType.Exp
    mult = mybir.AluOpType.mult
    add = mybir.AluOpType.add

    NQ = S_ // 512
    NK = S_ // 128
    NS = S_ // 128

    P = ctx.enter_context(tc.tile_pool(name="persist", bufs=1))

    wk_sb = P.tile([128, 512], bf16, tag="wk")
    nc.sync.dma_start(wk_sb[:], aps["wk"])
    bk_sb = P.tile([128, 1], f32, tag="bk")
    nc.sync.dma_start(bk_sb[:], aps["bk"])
    wq_sb = P.tile([128, 512], bf16, tag="wq")
    bq_sb = P.tile([128, 1], f32, tag="bq")
    wv_sb = P.tile([128, 512], bf16, tag="wv")
    wo_sb = P.tile([128, 512], bf16, tag="wo")

    qT = P.tile([128, S_], bf16, tag="qT")
    kT = P.tile([128, S_], bf16, tag="kT")
    Vt = P.tile([128, NK * 130], bf16, tag="V")
    ctxT = P.tile([128, S_], bf16, tag="ctxT")
    sumsT = [P.tile([128, NS], f32, tag=f"sumsT{h}", name=f"sumsT{h}")
             for h in (0, 1)]
    recT = [P.tile([128, NS], f32, tag=f"recT{h}", name=f"recT{h}")
            for h in (0, 1)]
    ones1 = P.tile([128, 1], f32, tag="ones1")
    nc.vector.memset(ones1[:], 1.0)
    vt_cols = Vt.rearrange("p (n c) -> p n c", c=65)
    nc.vector.tensor_copy(
        vt_cols[:, :, 64:65], ones1[:, 0:1].to_broadcast((128, 2 * NK, 1))
    )

    nmp = ctx.enter_context(tc.tile_pool(name="nm", bufs=4))
    drp = ctx.enter_context(tc.tile_pool(name="dr", bufs=4, space="DRAM"))
    pbp = ctx.enter_context(tc.tile_pool(name="pb", bufs=3))
    cxp = ctx.enter_context(tc.tile_pool(name="cx", bufs=2, space="PSUM"))

    def scores_mm(sc_slice, h, t, c):
        nc.tensor.matmul(
            sc_slice,
            kT[h * 64:(h + 1) * 64, t * 128:(t + 1) * 128],
            qT[h * 64:(h + 1) * 64, c * 512:(c + 1) * 512],
            start=True, stop=True,
        )

    def pv_mm(cxt, h, t, pb_slice):
        nc.tensor.matmul(
            cxt,
            Vt[:, t * 130 + h * 65:t * 130 + (h + 1) * 65],
            pb_slice,
            start=(t == 0), stop=(t == NK - 1),
        )

    def evac(cxm, h, c):
        sums = nmp.tile([1, 512], f32, tag="sums", name="sums")
        nc.vector.tensor_copy(sums[0:1, :], cxm[64:65, :])
        nc.vector.tensor_copy(
            ctxT[h * 64:(h + 1) * 64, c * 512:(c + 1) * 512], cxm[0:64, :]
        )
        dsc = drp.tile([1, 512], f32, tag="dsums", name="dsc")
        nc.sync.dma_start(dsc[:], sums[0:1, :])
        nc.sync.dma_start(
            sumsT[h][:, 4 * c:4 * c + 4],
            dsc.rearrange("o (j p) -> (o p) j", p=128),
        )
        nc.vector.reciprocal_approx_fast(
            recT[h][:, 4 * c:4 * c + 4], sumsT[h][:, 4 * c:4 * c + 4]
        )

    with tc.tile_pool(name="xt", bufs=1) as xtp, \
         tc.tile_pool(name="pj", bufs=2, space="PSUM") as pj, \
         tc.tile_pool(name="s2", bufs=2, space="PSUM") as s2p:
        xts = []
        for j in range(4):
            t = xtp.tile([128, S_], bf16, tag=f"xt{j}")
            xts.append(t)
        for j in range(4):
            nc.sync.dma_start(
                xts[j][:, 0:512], aps["xT"][j * 128:(j + 1) * 128, 0:512]
            )
        nc.sync.dma_start(wq_sb[:], aps["wq"])
        nc.sync.dma_start(bq_sb[:], aps["bq"])
        nc.sync.dma_start(wv_sb[:], aps["wv"])
        for c in range(1, NQ):
            for j in range(4):
                nc.sync.dma_start(
                    xts[j][:, c * 512:(c + 1) * 512],
                    aps["xT"][j * 128:(j + 1) * 128, c * 512:(c + 1) * 512],
                )
        nc.sync.dma_start(wo_sb[:], aps["wo"])

        def kq_proj_chunk(wsb, bsb, dst, c):
            ps = pj.tile([128, 512], f32, tag="pj", name="pjt")
            for j in range(4):
                nc.tensor.matmul(
                    ps[:],
                    wsb[:, j * 128:(j + 1) * 128],
                    xts[j][:, c * 512:(c + 1) * 512],
                    start=(j == 0), stop=(j == 3),
                )
            nc.vector.tensor_scalar_add(
                dst[:, c * 512:(c + 1) * 512], ps[:], bsb[:, 0:1]
            )

        def v_proj_tile(t):
            ps = pj.tile([128, 128], f32, tag="pj", name="pjv")
            for j in range(4):
                nc.tensor.matmul(
                    ps[:],
                    xts[j][:, t * 128:(t + 1) * 128],
                    wv_sb[:, j * 128:(j + 1) * 128],
                    start=(j == 0), stop=(j == 3),
                )
            nc.vector.tensor_copy(Vt[:, t * 130:t * 130 + 64], ps[:, 0:64])
            nc.vector.tensor_copy(
                Vt[:, t * 130 + 65:t * 130 + 129], ps[:, 64:128]
            )

        kq_proj_chunk(wk_sb, bk_sb, kT, 0)
        kq_proj_chunk(wq_sb, bq_sb, qT, 0)
        NVUP = min(4, NK)
        for t in range(NVUP):
            v_proj_tile(t)

        tasks = []
        vq = list(range(NVUP, NK))
        kq = list(range(1, NQ))
        while vq or kq:
            for _ in range(4):
                if vq:
                    tasks.append(("v", vq.pop(0)))
            if kq:
                tasks.append(("k", kq.pop(0)))
        tasks += [("q", c) for c in range(1, NQ)]
        ti = 0

        cx0 = {h: cxp.tile([65, 512], f32, tag="cx", name=f"cx{h}")
               for h in (0, 1)}
        for t in range(NK):
            sc = s2p.tile([128, 1024], f32, tag="s2", name="s2")
            scores_mm(sc[:, 0:512], 0, t, 0)
            scores_mm(sc[:, 512:1024], 1, t, 0)
            pb = pbp.tile([128, 1024], bf16, tag="pb", name="pb")
            nc.scalar.activation(pb[:], sc[:], Exp, scale=1.0 / np.sqrt(HD))
            if dbg and t == 0:
                nc.sync.dma_start(dbg["pb0"], pb[:])
            while ti * NK < (t + 1) * len(tasks):
                kind, v = tasks[ti]
                ti += 1
                if kind == "k":
                    kq_proj_chunk(wk_sb, bk_sb, kT, v)
                elif kind == "q":
                    kq_proj_chunk(wq_sb, bq_sb, qT, v)
                else:
                    v_proj_tile(v)
            pv_mm(cx0[0][:], 0, t, pb[:, 0:512])
            pv_mm(cx0[1][:], 1, t, pb[:, 512:1024])
        for h in (0, 1):
            if dbg:
                cxd = nmp.tile([65, 512], f32, tag="cxd")
                nc.vector.tensor_copy(cxd[:], cx0[h][:])
                nc.sync.dma_start(dbg[f"cx{h}"], cxd[:])
            evac(cx0[h], h, 0)

    Ident = mybir.ActivationFunctionType.Identity

    def oproj_tile(st, opp, obp):
        ps0 = opp.tile([128, 512], f32, tag="op", name="ps0")
        nc.tensor.matmul(
            ps0[:],
            ctxT[0:64, st * 128:(st + 1) * 128],
            wo_sb[0:64, :],
            start=True, stop=True,
        )
        ps1 = opp.tile([128, 512], f32, tag="op", name="ps1")
        nc.tensor.matmul(
            ps1[:],
            ctxT[64:128, st * 128:(st + 1) * 128],
            wo_sb[64:128, :],
            start=True, stop=True,
        )
        tmp = obp.tile([128, 512], f32, tag="tmp")
        nc.scalar.activation(tmp[:], ps0[:], Ident,
                             scale=recT[0][:, st:st + 1])
        ob = obp.tile([128, 512], f32, tag="ob")
        nc.vector.scalar_tensor_tensor(
            ob[:], ps1[:], recT[1][:, st:st + 1], tmp[:],
            op0=mult, op1=add,
        )
        nc.sync.dma_start(aps["outp"][st * 128:(st + 1) * 128, :], ob[:])

    items = [(h, t) for t in range(NK) for h in (0, 1)]
    groups = [items[i:i + 3] for i in range(0, len(items), 3)]
    c_tail0 = max(1, NQ - 2)
    with tc.tile_pool(name="sc", bufs=2, space="PSUM") as scp:
        for c in range(1, c_tail0):
            cx = {h: cxp.tile([65, 512], f32, tag="cx", name=f"cx{h}")
                  for h in (0, 1)}
            for g in groups:
                n = len(g)
                sc = scp.tile([128, n * 512], f32, tag="sc")
                for i, (h, t) in enumerate(g):
                    scores_mm(sc[:, i * 512:(i + 1) * 512], h, t, c)
                pb = pbp.tile([128, n * 512], bf16, tag="pb", name="pb")
                nc.scalar.activation(pb[:], sc[:], Exp, scale=1.0 / np.sqrt(HD))
                for i, (h, t) in enumerate(g):
                    pv_mm(cx[h][:], h, t, pb[:, i * 512:(i + 1) * 512])
            for h in (0, 1):
                evac(cx[h], h, c)

    sts = list(range(NS))
    with tc.tile_pool(name="s3", bufs=2, space="PSUM") as s3p, \
         tc.tile_pool(name="op", bufs=2, space="PSUM") as opp, \
         tc.tile_pool(name="ob", bufs=3) as obp:
        for c in range(c_tail0, NQ):
            cx = {h: cxp.tile([65, 512], f32, tag="cx", name=f"cx{h}")
                  for h in (0, 1)}
            for t in range(NK):
                sc = s3p.tile([128, 1024], f32, tag="s3", name="s3")
                scores_mm(sc[:, 0:512], 0, t, c)
                scores_mm(sc[:, 512:1024], 1, t, c)
                pb = pbp.tile([128, 1024], bf16, tag="pb", name="pb")
                nc.scalar.activation(pb[:], sc[:], Exp, scale=1.0 / np.sqrt(HD))
                pv_mm(cx[0][:], 0, t, pb[:, 0:512])
                pv_mm(cx[1][:], 1, t, pb[:, 512:1024])
                if t % 2 == 1 and sts and sts[0] < 4 * c:
                    oproj_tile(sts.pop(0), opp, obp)
            for h in (0, 1):
                evac(cx[h], h, c)
        while sts:
            oproj_tile(sts.pop(0), opp, obp)

    if dbg:
        nc.sync.dma_start(dbg["qT"], qT[:])
        nc.sync.dma_start(dbg["kT"], kT[:])
        nc.sync.dma_start(dbg["Vt"], Vt[:])
        nc.sync.dma_start(dbg["ctxT0"], ctxT[0:64, :])
        nc.sync.dma_start(dbg["ctxT1"], ctxT[64:128, :])


def _build(S_=S, debug_dump=False):
    from contextlib import ExitStack

    import concourse.mybir as mybir
    import concourse.tile as tile
    from concourse import bacc

    f32 = mybir.dt.float32
    bf16 = mybir.dt.bfloat16
    nc = bacc.Bacc("TRN2", target_bir_lowering=False, debug=False,
                   num_devices=N_CORES)
    aps = {
        "xT": nc.dram_tensor("xT", [H, S_], bf16, kind="ExternalInput").ap(),
        "wq": nc.dram_tensor("wq", [128, H], bf16, kind="ExternalInput").ap(),
        "wk": nc.dram_tensor("wk", [128, H], bf16, kind="ExternalInput").ap(),
        "wv": nc.dram_tensor("wv", [128, H], bf16, kind="ExternalInput").ap(),
        "wo": nc.dram_tensor("wo", [128, H], bf16, kind="ExternalInput").ap(),
        "bq": nc.dram_tensor("bq", [128, 1], f32, kind="ExternalInput").ap(),
        "bk": nc.dram_tensor("bk", [128, 1], f32, kind="ExternalInput").ap(),
        "outp": nc.dram_tensor("outp", [S_, H], f32, kind="ExternalOutput").ap(),
    }
    dbg = None
    if debug_dump:
        NK = S_ // 128
        dbg = {
            "qT": nc.dram_tensor("d_qT", [128, S_], bf16,
                                 kind="ExternalOutput").ap(),
            "kT": nc.dram_tensor("d_kT", [128, S_], bf16,
                                 kind="ExternalOutput").ap(),
            "Vt": nc.dram_tensor("d_Vt", [128, NK * 130], bf16,
                                 kind="ExternalOutput").ap(),
            "ctxT0": nc.dram_tensor("d_ctxT0", [64, S_], bf16,
                                    kind="ExternalOutput").ap(),
            "ctxT1": nc.dram_tensor("d_ctxT1", [64, S_], bf16,
                                    kind="ExternalOutput").ap(),
            "pb0": nc.dram_tensor("d_pb0", [128, 2 * 512], bf16,
                                  kind="ExternalOutput").ap(),
            "cx0": nc.dram_tensor("d_cx0", [65, 512], f32,
                                  kind="ExternalOutput").ap(),
            "cx1": nc.dram_tensor("d_cx1", [65, 512], f32,
                                  kind="ExternalOutput").ap(),
        }
    with tile.TileContext(nc) as tc:
        with ExitStack() as stack:
            _emit(nc, tc, stack, aps, S_, dbg=dbg)
    nc.compile()
    return nc


def _rearrange_w(wT_slice):
    import ml_dtypes

    return np.ascontiguousarray(
        wT_slice.reshape(4, 128, 128).transpose(1, 0, 2).reshape(128, 512)
    ).astype(ml_dtypes.bfloat16)


def _host_prep(hidden_states, Wq, bq, Wk, bk, Wv, bv, Wo, bo, S_=S):
    import ml_dtypes

    bf = ml_dtypes.bfloat16
    xT = [np.ascontiguousarray(hidden_states[b].T).astype(bf)
          for b in range(B)]
    in_maps = []
    for c in range(N_CORES):
        b, p = c // 4, c % 4
        sl = slice(p * 128, (p + 1) * 128)
        in_maps.append({
            "xT": xT[b],
            "wq": _rearrange_w(Wq.T[:, sl].astype(np.float32)),
            "wk": _rearrange_w(Wk.T[:, sl].astype(np.float32)),
            "wv": _rearrange_w(Wv.T[:, sl].astype(np.float32)),
            "wo": np.ascontiguousarray(
                Wo.T[p * 128:(p + 1) * 128, :]).astype(bf),
            "bq": np.ascontiguousarray(
                bq[sl].astype(np.float32).reshape(128, 1)),
            "bk": np.ascontiguousarray(
                bk[sl].astype(np.float32).reshape(128, 1)),
        })
    return in_maps


def kernel(hidden_states, Wq, bq, Wk, bk, Wv, bv, Wo, bo):
    from concourse.bass_utils import run_bass_kernel_spmd

    hidden_states = np.asarray(hidden_states)
    Wq, bq = np.asarray(Wq), np.asarray(bq)
    Wk, bk = np.asarray(Wk), np.asarray(bk)
    Wv, bv = np.asarray(Wv), np.asarray(bv)
    Wo, bo = np.asarray(Wo), np.asarray(bo)

    if "nc" not in _CACHE:
        _CACHE["nc"] = _build(S)
    nc = _CACHE["nc"]

    in_maps = _host_prep(hidden_states, Wq, bq, Wk, bk, Wv, bv, Wo, bo)
    res = run_bass_kernel_spmd(nc, in_maps, core_ids=list(range(N_CORES)))

    bo_eff = (bo.astype(np.float64) +
              bv.astype(np.float64) @ Wo.T.astype(np.float64)).astype(np.float32)
    out = np.empty((B, S, H), dtype=np.float32)
    for b in range(B):
        acc = np.zeros((S, H), dtype=np.float32)
        for p in range(4):
            acc += res.results[b * 4 + p]["outp"]
        out[b] = acc + bo_eff
    return out


# revision 42
# speedup vs baseline: 1.1915x; 1.0490x over previous
import numpy as np

B, S, H, NH, HD = 2, 4096, 512, 8, 64
N_CORES = 8

_CACHE = {}


def _emit(nc, tc, ctx, aps, S_, dbg=None):
    import concourse.mybir as mybir

    f32 = mybir.dt.float32
    bf16 = mybir.dt.bfloat16
    Exp = mybir.ActivationFunctionType.Exp
    mult = mybir.AluOpType.mult
    add = mybir.AluOpType.add

    NQ = S_ // 512
    NK = S_ // 128
    NS = S_ // 128

    P = ctx.enter_context(tc.tile_pool(name="persist", bufs=1))

    wk_sb = P.tile([128, 512], bf16, tag="wk")
    nc.sync.dma_start(wk_sb[:], aps["wk"])
    bk_sb = P.tile([128, 1], f32, tag="bk")
    nc.sync.dma_start(bk_sb[:], aps["bk"])
    wq_sb = P.tile([128, 512], bf16, tag="wq")
    bq_sb = P.tile([128, 1], f32, tag="bq")
    wv_sb = P.tile([128, 512], bf16, tag="wv")
    wo_sb = P.tile([128, 512], bf16, tag="wo")

    qT = P.tile([128, S_], bf16, tag="qT")
    kT = P.tile([128, S_], bf16, tag="kT")
    Vt = P.tile([128, NK * 130], bf16, tag="V")
    ctxT = P.tile([128, S_], bf16, tag="ctxT")
    sumsT = [P.tile([128, NS], f32, tag=f"sumsT{h}", name=f"sumsT{h}")
             for h in (0, 1)]
    recT = [P.tile([128, NS], f32, tag=f"recT{h}", name=f"recT{h}")
            for h in (0, 1)]
    ones1 = P.tile([128, 1], f32, tag="ones1")
    nc.vector.memset(ones1[:], 1.0)
    vt_cols = Vt.rearrange("p (n c) -> p n c", c=65)
    nc.vector.tensor_copy(
        vt_cols[:, :, 64:65], ones1[:, 0:1].to_broadcast((128, 2 * NK, 1))
    )

    nmp = ctx.enter_context(tc.tile_pool(name="nm", bufs=4))
    drp = ctx.enter_context(tc.tile_pool(name="dr", bufs=4, space="DRAM"))
    pbp = ctx.enter_context(tc.tile_pool(name="pb", bufs=3))
    cxp = ctx.enter_context(tc.tile_pool(name="cx", bufs=2, space="PSUM"))

    def scores_mm(sc_slice, h, t, c):
        nc.tensor.matmul(
            sc_slice,
            kT[h * 64:(h + 1) * 64, t * 128:(t + 1) * 128],
            qT[h * 64:(h + 1) * 64, c * 512:(c + 1) * 512],
            start=True, stop=True,
        )

    def pv_mm(cxt, h, t, pb_slice):
        nc.tensor.matmul(
            cxt,
            Vt[:, t * 130 + h * 65:t * 130 + (h + 1) * 65],
            pb_slice,
            start=(t == 0), stop=(t == NK - 1),
        )

    def evac(cxm, h, c):
        sums = nmp.tile([1, 512], f32, tag="sums", name="sums")
        nc.vector.tensor_copy(sums[0:1, :], cxm[64:65, :])
        nc.vector.tensor_copy(
            ctxT[h * 64:(h + 1) * 64, c * 512:(c + 1) * 512], cxm[0:64, :]
        )
        dsc = drp.tile([1, 512], f32, tag="dsums", name="dsc")
        nc.sync.dma_start(dsc[:], sums[0:1, :])
        nc.sync.dma_start(
            sumsT[h][:, 4 * c:4 * c + 4],
            dsc.rearrange("o (j p) -> (o p) j", p=128),
        )
        nc.vector.reciprocal_approx_fast(
            recT[h][:, 4 * c:4 * c + 4], sumsT[h][:, 4 * c:4 * c + 4]
        )

    with tc.tile_pool(name="xt", bufs=1) as xtp, \
         tc.tile_pool(name="pj", bufs=2, space="PSUM") as pj, \
         tc.tile_pool(name="s2", bufs=2, space="PSUM") as s2p:
        xts = []
        for j in range(4):
            t = xtp.tile([128, S_], bf16, tag=f"xt{j}")
            xts.append(t)
        for j in range(4):
            nc.sync.dma_start(
                xts[j][:, 0:512], aps["xT"][j * 128:(j + 1) * 128, 0:512]
            )
        nc.sync.dma_start(wq_sb[:], aps["wq"])
        nc.sync.dma_start(bq_sb[:], aps["bq"])
        nc.sync.dma_start(wv_sb[:], aps["wv"])
        for c in range(1, NQ):
            for j in range(4):
                nc.sync.dma_start(
                    xts[j][:, c * 512:(c + 1) * 512],
                    aps["xT"][j * 128:(j + 1) * 128, c * 512:(c + 1) * 512],
                )
        nc.sync.dma_start(wo_sb[:], aps["wo"])

        def kq_proj_chunk(wsb, bsb, dst, c):
            ps = pj.tile([128, 512], f32, tag="pj", name="pjt")
            for j in range(4):
                nc.tensor.matmul(
                    ps[:],
                    wsb[:, j * 128:(j + 1) * 128],
                    xts[j][:, c * 512:(c + 1) * 512],
                    start=(j == 0), stop=(j == 3),
                )
            nc.vector.tensor_scalar_add(
                dst[:, c * 512:(c + 1) * 512], ps[:], bsb[:, 0:1]
            )

        def v_proj_tile(t):
            ps = pj.tile([128, 128], f32, tag="pj", name="pjv")
            for j in range(4):
                nc.tensor.matmul(
                    ps[:],
                    xts[j][:, t * 128:(t + 1) * 128],
                    wv_sb[:, j * 128:(j + 1) * 128],
                    start=(j == 0), stop=(j == 3),
                )
            nc.vector.tensor_copy(Vt[:, t * 130:t * 130 + 64], ps[:, 0:64])
            nc.vector.tensor_copy(
                Vt[:, t * 130 + 65:t * 130 + 129], ps[:, 64:128]
            )

        kq_proj_chunk(wk_sb, bk_sb, kT, 0)
        kq_proj_chunk(wq_sb, bq_sb, qT, 0)
        NVUP = min(4, NK)
        for t in range(NVUP):
            v_proj_tile(t)

        tasks = []
        vq = list(range(NVUP, NK))
        kq = list(range(1, NQ))
        while vq or kq:
            for _ in range(4):
                if vq:
                    tasks.append(("v", vq.pop(0)))
            if kq:
                tasks.append(("k", kq.pop(0)))
        tasks += [("q", c) for c in range(1, NQ)]
        ti = 0

        cx0 = {h: cxp.tile([65, 512], f32, tag="cx", name=f"cx{h}")
               for h in (0, 1)}
        for t in range(NK):
            sc = s2p.tile([128, 1024], f32, tag="s2", name="s2")
            scores_mm(sc[:, 0:512], 0, t, 0)
            scores_mm(sc[:, 512:1024], 1, t, 0)
            pb = pbp.tile([128, 1024], bf16, tag="pb", name="pb")
            nc.scalar.activation(pb[:], sc[:], Exp, scale=1.0 / np.sqrt(HD))
            if dbg and t == 0:
                nc.sync.dma_start(dbg["pb0"], pb[:])
            while ti * NK < (t + 1) * len(tasks):
                kind, v = tasks[ti]
                ti += 1
                if kind == "k":
                    kq_proj_chunk(wk_sb, bk_sb, kT, v)
                elif kind == "q":
                    kq_proj_chunk(wq_sb, bq_sb, qT, v)
                else:
                    v_proj_tile(v)
            pv_mm(cx0[0][:], 0, t, pb[:, 0:512])
            pv_mm(cx0[1][:], 1, t, pb[:, 512:1024])
        for h in (0, 1):
            if dbg:
                cxd = nmp.tile([65, 512], f32, tag="cxd")
                nc.vector.tensor_copy(cxd[:], cx0[h][:])
                nc.sync.dma_start(dbg[f"cx{h}"], cxd[:])
            evac(cx0[h], h, 0)

    Ident = mybir.ActivationFunctionType.Identity

    def oproj_tile(st, opp, obp, use_act=True):
        ps0 = opp.tile([128, 512], f32, tag="op", name="ps0")
        nc.tensor.matmul(
            ps0[:],
            ctxT[0:64, st * 128:(st + 1) * 128],
            wo_sb[0:64, :],
            start=True, stop=True,
        )
        ps1 = opp.tile([128, 512], f32, tag="op", name="ps1")
        nc.tensor.matmul(
            ps1[:],
            ctxT[64:128, st * 128:(st + 1) * 128],
            wo_sb[64:128, :],
            start=True, stop=True,
        )
        tmp = obp.tile([128, 512], f32, tag="tmp")
        if use_act:
            nc.scalar.activation(tmp[:], ps0[:], Ident,
                                 scale=recT[0][:, st:st + 1])
        else:
            nc.vector.tensor_scalar_mul(tmp[:], ps0[:], recT[0][:, st:st + 1])
        ob = obp.tile([128, 512], f32, tag="ob")
        nc.vector.scalar_tensor_tensor(
            ob[:], ps1[:], recT[1][:, st:st + 1], tmp[:],
            op0=mult, op1=add,
        )
        nc.sync.dma_start(aps["outp"][st * 128:(st + 1) * 128, :], ob[:])

    items = [(h, t) for t in range(NK) for h in (0, 1)]
    groups = [items[i:i + 3] for i in range(0, len(items), 3)]
    c_tail0 = max(1, NQ - 2)
    with tc.tile_pool(name="sc", bufs=2, space="PSUM") as scp:
        for c in range(1, c_tail0):
            cx = {h: cxp.tile([65, 512], f32, tag="cx", name=f"cx{h}")
                  for h in (0, 1)}
            for g in groups:
                n = len(g)
                sc = scp.tile([128, n * 512], f32, tag="sc")
                for i, (h, t) in enumerate(g):
                    scores_mm(sc[:, i * 512:(i + 1) * 512], h, t, c)
                pb = pbp.tile([128, n * 512], bf16, tag="pb", name="pb")
                nc.scalar.activation(pb[:], sc[:], Exp, scale=1.0 / np.sqrt(HD))
                for i, (h, t) in enumerate(g):
                    pv_mm(cx[h][:], h, t, pb[:, i * 512:(i + 1) * 512])
            for h in (0, 1):
                evac(cx[h], h, c)

    sts = list(range(NS))
    with tc.tile_pool(name="s3", bufs=2, space="PSUM") as s3p, \
         tc.tile_pool(name="op", bufs=2, space="PSUM") as opp, \
         tc.tile_pool(name="ob", bufs=3) as obp:
        for c in range(c_tail0, NQ):
            cx = {h: cxp.tile([65, 512], f32, tag="cx", name=f"cx{h}")
                  for h in (0, 1)}
            for t in range(NK):
                sc = s3p.tile([128, 1024], f32, tag="s3", name="s3")
                scores_mm(sc[:, 0:512], 0, t, c)
                scores_mm(sc[:, 512:1024], 1, t, c)
                pb = pbp.tile([128, 1024], bf16, tag="pb", name="pb")
                nc.scalar.activation(pb[:], sc[:], Exp, scale=1.0 / np.sqrt(HD))
                pv_mm(cx[0][:], 0, t, pb[:, 0:512])
                pv_mm(cx[1][:], 1, t, pb[:, 512:1024])
                if t % 2 == 1 and sts and sts[0] < 4 * c:
                    oproj_tile(sts.pop(0), opp, obp, use_act=False)
            for h in (0, 1):
                evac(cx[h], h, c)
        while sts:
            oproj_tile(sts.pop(0), opp, obp)

    if dbg:
        nc.sync.dma_start(dbg["qT"], qT[:])
        nc.sync.dma_start(dbg["kT"], kT[:])
        nc.sync.dma_start(dbg["Vt"], Vt[:])
        nc.sync.dma_start(dbg["ctxT0"], ctxT[0:64, :])
        nc.sync.dma_start(dbg["ctxT1"], ctxT[64:128, :])


def _build(S_=S, debug_dump=False):
    from contextlib import ExitStack

    import concourse.mybir as mybir
    import concourse.tile as tile
    from concourse import bacc

    f32 = mybir.dt.float32
    bf16 = mybir.dt.bfloat16
    nc = bacc.Bacc("TRN2", target_bir_lowering=False, debug=False,
                   num_devices=N_CORES)
    aps = {
        "xT": nc.dram_tensor("xT", [H, S_], bf16, kind="ExternalInput").ap(),
        "wq": nc.dram_tensor("wq", [128, H], bf16, kind="ExternalInput").ap(),
        "wk": nc.dram_tensor("wk", [128, H], bf16, kind="ExternalInput").ap(),
        "wv": nc.dram_tensor("wv", [128, H], bf16, kind="ExternalInput").ap(),
        "wo": nc.dram_tensor("wo", [128, H], bf16, kind="ExternalInput").ap(),
        "bq": nc.dram_tensor("bq", [128, 1], f32, kind="ExternalInput").ap(),
        "bk": nc.dram_tensor("bk", [128, 1], f32, kind="ExternalInput").ap(),
        "outp": nc.dram_tensor("outp", [S_, H], f32, kind="ExternalOutput").ap(),
    }
    dbg = None
    if debug_dump:
        NK = S_ // 128
        dbg = {
            "qT": nc.dram_tensor("d_qT", [128, S_], bf16,
                                 kind="ExternalOutput").ap(),
            "kT": nc.dram_tensor("d_kT", [128, S_], bf16,
                                 kind="ExternalOutput").ap(),
            "Vt": nc.dram_tensor("d_Vt", [128, NK * 130], bf16,
                                 kind="ExternalOutput").ap(),
            "ctxT0": nc.dram_tensor("d_ctxT0", [64, S_], bf16,
                                    kind="ExternalOutput").ap(),
            "ctxT1": nc.dram_tensor("d_ctxT1", [64, S_], bf16,
                                    kind="ExternalOutput").ap(),
            "pb0": nc.dram_tensor("d_pb0", [128, 2 * 512], bf16,
                                  kind="ExternalOutput").ap(),
            "cx0": nc.dram_tensor("d_cx0", [65, 512], f32,
                                  kind="ExternalOutput").ap(),
            "cx1": nc.dram_tensor("d_cx1", [65, 512], f32,
                                  kind="ExternalOutput").ap(),
        }
    with tile.TileContext(nc) as tc:
        with ExitStack() as stack:
            _emit(nc, tc, stack, aps, S_, dbg=dbg)
    nc.compile()
    return nc


def _rearrange_w(wT_slice):
    import ml_dtypes

    return np.ascontiguousarray(
        wT_slice.reshape(4, 128, 128).transpose(1, 0, 2).reshape(128, 512)
    ).astype(ml_dtypes.bfloat16)


def _host_prep(hidden_states, Wq, bq, Wk, bk, Wv, bv, Wo, bo, S_=S):
    import ml_dtypes

    bf = ml_dtypes.bfloat16
    xT = [np.ascontiguousarray(hidden_states[b].T).astype(bf)
          for b in range(B)]
    in_maps = []
    for c in range(N_CORES):
        b, p = c // 4, c % 4
        sl = slice(p * 128, (p + 1) * 128)
        in_maps.append({
            "xT": xT[b],
            "wq": _rearrange_w(Wq.T[:, sl].astype(np.float32)),
            "wk": _rearrange_w(Wk.T[:, sl].astype(np.float32)),
            "wv": _rearrange_w(Wv.T[:, sl].astype(np.float32)),
            "wo": np.ascontiguousarray(
                Wo.T[p * 128:(p + 1) * 128, :]).astype(bf),
            "bq": np.ascontiguousarray(
                bq[sl].astype(np.float32).reshape(128, 1)),
            "bk": np.ascontiguousarray(
                bk[sl].astype(np.float32).reshape(128, 1)),
        })
    return in_maps


def kernel(hidden_states, Wq, bq, Wk, bk, Wv, bv, Wo, bo):
    from concourse.bass_utils import run_bass_kernel_spmd

    hidden_states = np.asarray(hidden_states)
    Wq, bq = np.asarray(Wq), np.asarray(bq)
    Wk, bk = np.asarray(Wk), np.asarray(bk)
    Wv, bv = np.asarray(Wv), np.asarray(bv)
    Wo, bo = np.asarray(Wo), np.asarray(bo)

    if "nc" not in _CACHE:
        _CACHE["nc"] = _build(S)
    nc = _CACHE["nc"]

    in_maps = _host_prep(hidden_states, Wq, bq, Wk, bk, Wv, bv, Wo, bo)
    res = run_bass_kernel_spmd(nc, in_maps, core_ids=list(range(N_CORES)))

    bo_eff = (bo.astype(np.float64) +
              bv.astype(np.float64) @ Wo.T.astype(np.float64)).astype(np.float32)
    out = np.empty((B, S, H), dtype=np.float32)
    for b in range(B):
        acc = np.zeros((S, H), dtype=np.float32)
        for p in range(4):
            acc += res.results[b * 4 + p]["outp"]
        out[b] = acc + bo_eff
    return out
